# revision 12
# baseline (speedup 1.0000x reference)
"""BertAttention (B=2, S=2048, H=1024, 16 heads) on 8 trn2 NeuronCores.

Sharding: tensor-parallel over heads (2 heads/core). Each core computes
Q/K/V projections for its 2 heads over all tokens, full attention for
those heads, then per-head context vectors are exchanged with an
AllToAll (1MB/core, bf16) so each core ends up with the full 1024-dim
context for a 512-token slice. Output dense + residual + LayerNorm run
locally on that slice; the host concatenates the 8 slices.

Device notes:
  - All big matmuls are bf16 (full PE rate + fast weight load; fp32
    accumulation in PSUM). Residual/LayerNorm stay fp32.
  - Scores are computed transposed ([key, query]) so the softmax
    reduction is on the partition axis. exp() needs no max-subtraction
    (scores are O(1)).
  - The two heads' score matmuls are packed into the PE array with
    K=64 row tiling; the two heads' P@V matmuls are packed with M=64
    column tiling (tile_position (0,0)/(0,64)) so they run concurrently
    on disjoint column groups. Softmax denominators accumulate in a
    separate PSUM tile via M=1 ones-matmuls at columns 0/32.
  - V^T is produced directly as small [128tok,128feat] matmuls from the
    hidden chunks (no PE transpose pass).
  - Batch 1's QKV projection work is interleaved into batch 0's
    attention chunks (which are exp/ACT-bound) as PE filler.
  - q,k 1/sqrt(sqrt(d)) scaling is folded into the weights on host;
    b_out is folded into the residual on host.
  - LayerNorm rsqrt = exp(-0.5*ln(var+eps)) (hardware Sqrt tables are
    imprecise); Ln/Exp are batched to avoid ACT table thrashing.
"""

import numpy as np
import ml_dtypes

import concourse.bacc as bacc
import concourse.bass as bass
import concourse.bass_utils as bass_utils
import concourse.mybir as mybir
import concourse.tile as tile

B, S, H, NH, HD = 2, 2048, 1024, 16, 64
N_CORES = 8
HPC = NH // N_CORES          # heads per core: 2
F = HPC * HD                 # features per core: 128
TOK = B * S                  # 4096
TPC = TOK // N_CORES         # tokens per core: 512
EPS = 1e-12
NSC = S // 512               # 4 s-chunks per batch
NTB = S // 128               # 16 key blocks per batch

f32 = mybir.dt.float32
f32r = mybir.dt.float32r
bf16 = mybir.dt.bfloat16
AF = mybir.ActivationFunctionType
ALU = mybir.AluOpType
BF16 = ml_dtypes.bfloat16


def _wlayout(wt):
    """[H, F] (h-major) -> partition-major: partition p holds h rows
    p, 128+p, ... concatenated; returned flattened to the declared [H, F]."""
    hdim, fdim = wt.shape
    perm = wt.reshape(hdim // 128, 128, fdim).transpose(1, 0, 2)
    return np.ascontiguousarray(perm).reshape(hdim, fdim)


def _build(use_mask: bool):
    nc = bacc.Bacc("TRN2", target_bir_lowering=False, debug=False,
                   enable_asserts=True, num_devices=N_CORES)

    hid_t = nc.dram_tensor("hid_t", [B * H, S], bf16, kind="ExternalInput")
    wq_t = nc.dram_tensor("wq_t", [H, F], bf16, kind="ExternalInput")
    wk_t = nc.dram_tensor("wk_t", [H, F], bf16, kind="ExternalInput")
    wv_t = nc.dram_tensor("wv_t", [H, F], bf16, kind="ExternalInput")
    bq = nc.dram_tensor("bq", [F, 1], f32, kind="ExternalInput")
    bk = nc.dram_tensor("bk", [F, 1], f32, kind="ExternalInput")
    bvr = nc.dram_tensor("bvr", [1, F], bf16, kind="ExternalInput")
    mask_t = nc.dram_tensor("mask_t", [128, B * NTB], f32, kind="ExternalInput")
    wo_t = nc.dram_tensor("wo_t", [H, H], bf16, kind="ExternalInput")
    resid = nc.dram_tensor("resid", [TPC, H], f32, kind="ExternalInput")
    gamma_b = nc.dram_tensor("gamma_b", [128, H], f32, kind="ExternalInput")
    beta_b = nc.dram_tensor("beta_b", [128, H], f32, kind="ExternalInput")
    out = nc.dram_tensor("out", [TPC, H], f32, kind="ExternalOutput")

    with tile.TileContext(nc) as tc:
        with tc.tile_pool(name="const", bufs=1) as cst, \
             tc.tile_pool(name="w", bufs=1) as wp, \
             tc.tile_pool(name="hid", bufs=2) as hp, \
             tc.tile_pool(name="qk", bufs=2) as qp, \
             tc.tile_pool(name="e", bufs=4) as ep, \
             tc.tile_pool(name="cx", bufs=2) as cp, \
             tc.tile_pool(name="op", bufs=2) as op, \
             tc.tile_pool(name="op1", bufs=1) as op1, \
             tc.tile_pool(name="ps_s", bufs=2, space="PSUM") as ps_s, \
             tc.tile_pool(name="ps_c", bufs=1, space="PSUM") as ps_c, \
             tc.tile_pool(name="ps_d", bufs=1, space="PSUM") as ps_d, \
             tc.tile_pool(name="dram", bufs=1, space="DRAM") as dp:

            # ---- QKV weights / constants ----
            wq_sb = wp.tile([128, 8, F], bf16, tag="wq")
            wk_sb = wp.tile([128, 8, F], bf16, tag="wk")
            wv_sb = wp.tile([128, 8, F], bf16, tag="wv")
            nc.sync.dma_start(wq_sb[:], wq_t.ap().rearrange("(p hb) f -> p hb f", p=128))
            nc.sync.dma_start(wk_sb[:], wk_t.ap().rearrange("(p hb) f -> p hb f", p=128))
            nc.sync.dma_start(wv_sb[:], wv_t.ap().rearrange("(p hb) f -> p hb f", p=128))

            bq_sb = cst.tile([F, 1], f32, tag="bq")
            bk_sb = cst.tile([F, 1], f32, tag="bk")
            bvr_sb = cst.tile([1, F], bf16, tag="bvr")
            nc.gpsimd.dma_start(bq_sb[:], bq[:, :])
            nc.gpsimd.dma_start(bk_sb[:], bk[:, :])
            nc.gpsimd.dma_start(bvr_sb[:], bvr[:, :])
            onesr = cst.tile([1, 128], bf16, tag="onesr")
            nc.vector.memset(onesr[:, :], 1.0)
            ones128 = cst.tile([128, 1], bf16, tag="ones128")
            nc.vector.memset(ones128[:, :], 1.0)
            eps_sb = cst.tile([128, 1], f32, tag="eps")
            nc.vector.memset(eps_sb[:, :], EPS)
            warm = cst.tile([128, 1], f32, tag="warm")
            nc.scalar.activation(warm[:, :], eps_sb[:, :], AF.Exp)
            nc.scalar.activation(warm[:, :], warm[:, :], AF.Ln, bias=eps_sb[:, :])
            if use_mask:
                msk_sb = cst.tile([128, B * NTB], f32, tag="msk")
                nc.gpsimd.dma_start(msk_sb[:], mask_t[:, :])

            # A2A payload: 8 feature-major blocks [128 feat, 512 tok];
            # block j = my heads' normalized ctx^T for tokens [512j, 512j+512).
            a2a_in = dp.tile([N_CORES, 128, 512], bf16, tag="a2a_in")
            a2a_out = dp.tile([N_CORES, 128, 512], bf16, tag="a2a_out")

            # ---- per-batch state ----
            def alloc_batch(b):
                return {
                    "QT": qp.tile([128, S], bf16, tag="qt", name=f"qt_{b}"),
                    "KT": qp.tile([128, S], bf16, tag="kt", name=f"kt_{b}"),
                    "Vpl": [qp.tile([128, NTB, HD], bf16, tag=f"vpl_{h}",
                                    name=f"vpl_{b}_{h}") for h in range(HPC)],
                }

            def hch_dma(b, sc):
                hrows = hid_t.ap()[b * H:(b + 1) * H, :].rearrange(
                    "(hb p) s -> p hb s", p=128)
                hch4 = [hp.tile([128, 2, 512], bf16, tag=f"h{q}",
                                name=f"hch_{b}_{sc}_{q}") for q in range(4)]
                for q in range(4):
                    nc.sync.dma_start(
                        hch4[q][:],
                        hrows[:, 2 * q:2 * q + 2, sc * 512:(sc + 1) * 512])
                return hch4

            def qk_wide(b, sc, st, hch4):
                """Q and K projections for one s-chunk through a [128,2,512] psum."""
                qk_ps = ps_s.tile([128, 2, 512], f32, tag="s", name=f"qkps_{b}_{sc}")
                for hb in range(8):
                    nc.tensor.matmul(qk_ps[:, 0:1, :], wq_sb[:, hb:hb + 1, :],
                                     hch4[hb // 2][:, hb % 2:hb % 2 + 1, :],
                                     start=(hb == 0), stop=(hb == 7),
                                     skip_group_check=True)
                for hb in range(8):
                    nc.tensor.matmul(qk_ps[:, 1:2, :], wk_sb[:, hb:hb + 1, :],
                                     hch4[hb // 2][:, hb % 2:hb % 2 + 1, :],
                                     start=(hb == 0), stop=(hb == 7),
                                     skip_group_check=True)
                sl = slice(sc * 512, (sc + 1) * 512)
                nc.vector.tensor_scalar_add(st["QT"][:, sl], qk_ps[:, 0:1, :], bq_sb[:, :])
                nc.vector.tensor_scalar_add(st["KT"][:, sl], qk_ps[:, 1:2, :], bk_sb[:, :])

            def proj_seq(b, sc, st, hch4, which):
                """One projection (q or k) through the small 'v' psum slot."""
                p_ps = ps_c.tile([128, 512], f32, tag="v", name=f"p_{b}_{sc}_{which}")
                w_sb = wq_sb if which == "q" else wk_sb
                for hb in range(8):
                    nc.tensor.matmul(p_ps[:, :], w_sb[:, hb:hb + 1, :],
                                     hch4[hb // 2][:, hb % 2:hb % 2 + 1, :],
                                     start=(hb == 0), stop=(hb == 7),
                                     skip_group_check=True)
                sl = slice(sc * 512, (sc + 1) * 512)
                dst = st["QT"] if which == "q" else st["KT"]
                b_sb = bq_sb if which == "q" else bk_sb
                nc.vector.tensor_scalar_add(dst[:, sl], p_ps[:, :], b_sb[:, :])

            def vblock(b, sc, j, st, hch4):
                """V^T for one 128-token block: [128 tok, 128 feat] psum."""
                tb = sc * 4 + j
                vd = ps_d.tile([128, 128], f32, tag="vd", name=f"vd_{b}_{tb}")
                tsl = slice(j * 128, (j + 1) * 128)
                for hb in range(8):
                    nc.tensor.matmul(vd[:, :],
                                     hch4[hb // 2][:, hb % 2, tsl],
                                     wv_sb[:, hb:hb + 1, :],
                                     start=(hb == 0), stop=False,
                                     skip_group_check=True)
                nc.tensor.matmul(vd[:, :], onesr[:, :], bvr_sb[:, :],
                                 start=False, stop=True, skip_group_check=True)
                for h in range(HPC):
                    nc.vector.tensor_copy(st["Vpl"][h][:, tb:tb + 1, :],
                                          vd[:, h * HD:(h + 1) * HD])

            def attn_chunk(b, sc, st, filler):
                QT, KT, Vpl = st["QT"], st["KT"], st["Vpl"]
                ssl = slice(sc * 512, (sc + 1) * 512)
                ctx_ps = ps_c.tile([128, 512], f32, tag="cA", name=f"ctx_{b}_{sc}")
                den_ps = ps_c.tile([33, 512], f32, tag="den", name=f"den_{b}_{sc}")
                es = {}

                def scores_exp(tb):
                    tsl = slice(tb * 128, (tb + 1) * 128)
                    sc_ps = ps_s.tile([128, 2, 512], f32, tag="s",
                                      name=f"scps_{b}_{sc}_{tb}")
                    e_sb = ep.tile([128, 2, 512], bf16, tag="e",
                                   name=f"e_{b}_{sc}_{tb}")
                    nc.tensor.matmul(sc_ps[:, 0:1, :], KT[0:64, tsl], QT[0:64, ssl],
                                     start=True, stop=True, skip_group_check=True,
                                     tile_position=(0, 0))
                    nc.tensor.matmul(sc_ps[:, 1:2, :], KT[64:128, tsl], QT[64:128, ssl],
                                     start=True, stop=True, skip_group_check=True,
                                     tile_position=(64, 0))
                    if use_mask:
                        for h in range(HPC):
                            nc.scalar.activation(
                                e_sb[:, h:h + 1, :], sc_ps[:, h:h + 1, :], AF.Exp,
                                bias=msk_sb[:, b * NTB + tb:b * NTB + tb + 1])
                    else:
                        nc.scalar.activation(e_sb[:, :, :], sc_ps[:, :, :], AF.Exp)
                    es[tb] = e_sb

                def ctx_den(tb):
                    e_sb = es.pop(tb)
                    st_f, sp_f = (tb == 0), (tb == NTB - 1)
                    nc.tensor.matmul(ctx_ps[0:64, :], Vpl[0][:, tb:tb + 1, :],
                                     e_sb[:, 0:1, :], start=st_f, stop=sp_f,
                                     skip_group_check=True, tile_position=(0, 0))
                    nc.tensor.matmul(ctx_ps[64:128, :], Vpl[1][:, tb:tb + 1, :],
                                     e_sb[:, 1:2, :], start=st_f, stop=sp_f,
                                     skip_group_check=True, tile_position=(0, 64))
                    nc.tensor.matmul(den_ps[0:1, :], ones128[:, :],
                                     e_sb[:, 0:1, :], start=st_f, stop=sp_f,
                                     skip_group_check=True, tile_position=(0, 0))
                    nc.tensor.matmul(den_ps[32:33, :], ones128[:, :],
                                     e_sb[:, 1:2, :], start=st_f, stop=sp_f,
                                     skip_group_check=True, tile_position=(0, 32))

                scores_exp(0)
                scores_exp(1)
                for g in range(1, NTB // 2):
                    scores_exp(2 * g)
                    scores_exp(2 * g + 1)
                    ctx_den(2 * g - 2)
                    ctx_den(2 * g - 1)
                    filler()
                ctx_den(NTB - 2)
                ctx_den(NTB - 1)
                filler()
                # evacuate, normalize, stage the A2A block
                cstg = cp.tile([128, 512], f32, tag="ctxa", name=f"cstg_{b}_{sc}")
                nc.vector.tensor_copy(cstg[:, :], ctx_ps[:, :])
                r0 = cp.tile([1, 512], f32, tag="r0")
                r1 = cp.tile([1, 512], f32, tag="r1")
                nc.vector.reciprocal(r0[:, :], den_ps[0:1, :])
                nc.vector.reciprocal(r1[:, :], den_ps[32:33, :])
                rbc = cp.tile([128, 512], f32, tag="rbc")
                rb1 = cp.tile([64, 512], f32, tag="rb1")
                nc.gpsimd.partition_broadcast(rbc[0:64, :], r0[:, :])
                nc.gpsimd.partition_broadcast(rb1[:, :], r1[:, :])
                nc.vector.tensor_copy(rbc[64:128, :], rb1[:, :])
                nstg = cp.tile([128, 512], bf16, tag="nstg", name=f"nstg_{b}_{sc}")
                nc.vector.tensor_tensor(nstg[:, :], cstg[:, :], rbc[:, :],
                                        op=ALU.mult)
                nc.sync.dma_start(a2a_in[4 * b + sc, :, :], nstg[:, :])

            # ---- batch 0 QKV ----
            st0 = alloc_batch(0)
            for sc in range(NSC):
                hch4 = hch_dma(0, sc)
                qk_wide(0, sc, st0, hch4)
                for j in range(4):
                    vblock(0, sc, j, st0, hch4)

            # output-phase operands: prefetch during batch-0 attention
            wo_sb = wp.tile([128, 8, H], bf16, tag="wo")
            nc.sync.dma_start(wo_sb[:], wo_t.ap().rearrange("(p fb) o -> p fb o", p=128))
            gam_sb = cst.tile([128, H], f32, tag="gam")
            bet_sb = cst.tile([128, H], f32, tag="bet")
            nc.gpsimd.dma_start(gam_sb[:], gamma_b[:, :])
            nc.gpsimd.dma_start(bet_sb[:], beta_b[:, :])
            res_all = wp.tile([128, 4, H], f32, tag="res")
            nc.sync.dma_start(res_all[:], resid.ap().rearrange("(sb p) h -> p sb h", p=128))

            # ---- batch 1 QKV, emitted as filler inside batch-0 attention ----
            st1 = alloc_batch(1)
            b1_hch = {}

            def b1_units():
                for sc in range(NSC):
                    def dma_u(sc=sc):
                        b1_hch[sc] = hch_dma(1, sc)
                    yield dma_u
                    yield lambda sc=sc: proj_seq(1, sc, st1, b1_hch[sc], "q")
                    yield lambda sc=sc: proj_seq(1, sc, st1, b1_hch[sc], "k")
                    for j in range(4):
                        yield lambda sc=sc, j=j: vblock(1, sc, j, st1, b1_hch[sc])

            units = b1_units()

            def filler():
                try:
                    next(units)()
                except StopIteration:
                    pass

            def no_fill():
                pass

            for sc in range(NSC):
                attn_chunk(0, sc, st0, filler)
            # drain any remaining batch-1 units
            for u in units:
                u()
            for sc in range(NSC):
                attn_chunk(1, sc, st1, no_fill)

            # ---- exchange context slices ----
            nc.gpsimd.collective_compute(
                "AllToAll", ALU.bypass,
                replica_groups=[list(range(N_CORES))],
                ins=[a2a_in.opt()], outs=[a2a_out.opt()])

            # ---- output dense + residual + LayerNorm on my 512 tokens ----
            recvT = op1.tile([128, 8, 512], bf16, tag="recvT")
            ro = a2a_out.rearrange("j p s -> p j s")
            for q in range(4):
                nc.sync.dma_start(recvT[:, 2 * q:2 * q + 2, :],
                                  ro[:, 2 * q:2 * q + 2, :])
            vacc = op1.tile([128, 4], f32, tag="vacc")
            rstd = op1.tile([128, 4], f32, tag="rstd")
            xcs = []
            for sb in range(4):
                o_ps = ps_s.tile([128, 2, 512], f32, tag="s", name=f"ops_{sb}")
                bsl = slice(sb * 128, (sb + 1) * 128)
                for oc in range(2):
                    for j in range(N_CORES):
                        nc.tensor.matmul(
                            o_ps[:, oc:oc + 1, :],
                            recvT[:, j:j + 1, bsl],
                            wo_sb[:, j:j + 1, oc * 512:(oc + 1) * 512],
                            start=(j == 0), stop=(j == N_CORES - 1),
                            skip_group_check=True)
                x = op.tile([128, H], f32, tag="x", name=f"x_{sb}")
                acc = op.tile([128, 2], f32, tag="acc")
                for oc in range(2):
                    nc.vector.scalar_tensor_tensor(
                        out=x[:, oc * 512:(oc + 1) * 512],
                        in0=o_ps[:, oc:oc + 1, :], scalar=1.0,
                        in1=res_all[:, sb, oc * 512:(oc + 1) * 512],
                        op0=ALU.mult, op1=ALU.add,
                        accum_out=acc[:, oc:oc + 1])
                mean = op.tile([128, 1], f32, tag="mean")
                nc.vector.tensor_add(mean[:, :], acc[:, 0:1], acc[:, 1:2])
                nc.vector.tensor_scalar_mul(mean[:, :], mean[:, :], 1.0 / H)
                xc = op1.tile([128, H], f32, tag=f"xc{sb}", name=f"xc_{sb}")
                nc.vector.tensor_scalar_sub(xc[:, :], x[:, :], mean[:, :])
                nc.vector.scalar_tensor_tensor(
                    out=x[:, :], in0=xc[:, :], scalar=1.0, in1=xc[:, :],
                    op0=ALU.mult, op1=ALU.mult, accum_out=vacc[:, sb:sb + 1])
                xcs.append(xc)
                if sb % 2 == 1:
                    hs = slice(sb - 1, sb + 1)
                    lnv = op1.tile([128, 2], f32, tag=f"lnv{sb}", name=f"lnv_{sb}")
                    nc.scalar.activation(lnv[:, :], vacc[:, hs], AF.Ln,
                                         bias=eps_sb[:, :], scale=1.0 / H)
                    nc.scalar.activation(rstd[:, hs], lnv[:, :], AF.Exp, scale=-0.5)
            for sb in range(4):
                y = op.tile([128, H], f32, tag="y")
                nc.vector.scalar_tensor_tensor(
                    out=y[:, :], in0=xcs[sb][:, :], scalar=rstd[:, sb:sb + 1],
                    in1=gam_sb[:, :], op0=ALU.mult, op1=ALU.mult)
                nc.vector.tensor_add(y[:, :], y[:, :], bet_sb[:, :])
                nc.sync.dma_start(out[sb * 128:(sb + 1) * 128, :], y[:, :])

    nc.compile()
    return nc


_NC_CACHE = {}


def _get_nc(use_mask: bool):
    if use_mask not in _NC_CACHE:
        _NC_CACHE[use_mask] = _build(use_mask)
    return _NC_CACHE[use_mask]


def _prep_in_maps(hidden_states, attention_mask, W_qkv, b_qkv, W_out, b_out,
                  ln_gamma, ln_beta):
    hidden_states = np.asarray(hidden_states, dtype=np.float32)
    attention_mask = np.asarray(attention_mask, dtype=np.float32)
    W_qkv = np.asarray(W_qkv, dtype=np.float32)
    b_qkv = np.asarray(b_qkv, dtype=np.float32)
    W_out = np.asarray(W_out, dtype=np.float32)
    b_out = np.asarray(b_out, dtype=np.float32)
    ln_gamma = np.asarray(ln_gamma, dtype=np.float32)
    ln_beta = np.asarray(ln_beta, dtype=np.float32)

    sc2 = np.float32(1.0 / np.sqrt(np.float32(HD)))   # (1/d^0.25)^2

    hid_t = np.ascontiguousarray(
        hidden_states.transpose(0, 2, 1).reshape(B * H, S)).astype(BF16)
    # mask[b, t] -> [128, B*NTB]: column b*NTB+tb = mask for key block tb
    mask_bt = attention_mask.reshape(B, S)
    mask_tiles = np.ascontiguousarray(
        mask_bt.reshape(B, NTB, 128).transpose(2, 0, 1).reshape(128, B * NTB))
    hid_flat = hidden_states.reshape(TOK, H)
    gamma_b = np.ascontiguousarray(np.broadcast_to(ln_gamma, (128, H)))
    beta_b = np.ascontiguousarray(np.broadcast_to(ln_beta, (128, H)))
    wo_l = _wlayout(W_out.T).astype(BF16)

    in_maps = []
    for c in range(N_CORES):
        r0 = c * F
        wq = W_qkv[r0:r0 + F, :]
        wk = W_qkv[H + r0:H + r0 + F, :]
        wv = W_qkv[2 * H + r0:2 * H + r0 + F, :]
        in_maps.append({
            "hid_t": hid_t,
            "wq_t": _wlayout(wq.T * sc2).astype(BF16),
            "wk_t": _wlayout(wk.T).astype(BF16),
            "wv_t": _wlayout(wv.T).astype(BF16),
            "bq": np.ascontiguousarray((b_qkv[r0:r0 + F] * sc2).reshape(F, 1)),
            "bk": np.ascontiguousarray(b_qkv[H + r0:H + r0 + F].reshape(F, 1)),
            "bvr": np.ascontiguousarray(
                b_qkv[2 * H + r0:2 * H + r0 + F].reshape(1, F)).astype(BF16),
            "mask_t": mask_tiles,
            "wo_t": wo_l,
            "resid": np.ascontiguousarray(hid_flat[c * TPC:(c + 1) * TPC, :] + b_out),
            "gamma_b": gamma_b,
            "beta_b": beta_b,
        })
    return in_maps


def run(trace=False, **inputs):
    use_mask = bool(np.any(np.asarray(inputs["attention_mask"])))
    nc = _get_nc(use_mask)
    in_maps = _prep_in_maps(**inputs)
    res = bass_utils.run_bass_kernel_spmd(
        nc, in_maps, core_ids=list(range(N_CORES)), trace=trace)
    out = np.concatenate([res.results[c]["out"] for c in range(N_CORES)], axis=0)
    return out.reshape(B, S, H).astype(np.float32), res


def kernel(**inputs):
    out, _ = run(trace=False, **inputs)
    return out
